# revision 25
# baseline (speedup 1.0000x reference)
"""Causal self-attention Trainium2 kernel (8 NeuronCores, SPMD).

Sharding: data-parallel over batch (B=2) x tensor-parallel over heads
(16 heads -> 4 per core).  core c: batch c//4, heads 4*(c%4) .. +4.
Each core computes qkv projection for its heads, causal attention, and a
partial out-projection; the host sums the 4 head-group partials per batch.

Layout notes:
  - Host passes x[b] pre-transposed (C, T) so the contraction dim C is
    partition-major for the qkv matmuls (PE contracts over partitions).
  - q,k are produced transposed (head_dim, T); v in natural (T, head_dim)
    with a ones column appended so the AV matmul also emits softmax row
    sums (row 64 of the PSUM accumulator).
  - scores are computed transposed (k, q) so the exp'd weights feed the
    AV matmul directly as the moving operand.
  - All matmul operands are bitcast to float32r (full-rate fp32 PE mode).
"""

import numpy as np

import concourse.bass as bass
import concourse.mybir as mybir
import concourse.tile as tile
from concourse import bacc
from concourse import bass_utils

# Problem shape (hardcoded per spec)
B, T, C = 2, 2048, 1024
NH, HD = 16, 64
NCORES = 8
HPC = 4                      # heads per core
P = 128                      # partitions
CB = C // P                  # 8 contraction blocks
QCW = 512                    # query chunk width
NQC = T // QCW               # 4 query chunks
NKB = T // P                 # 16 key blocks
SCALE = 1.0 / 8.0            # 1/sqrt(HD)

F32 = mybir.dt.float32
F32R = mybir.dt.float32r
EXP = mybir.ActivationFunctionType.Exp


def build_program():
    nc = bacc.Bacc("TRN2", target_bir_lowering=False, debug=False,
                   num_devices=NCORES)

    xT = nc.dram_tensor("xT", [C, T], F32R, kind="ExternalInput").ap()
    wqk = nc.dram_tensor("wqk", [C, 2 * HPC * HD], F32R, kind="ExternalInput").ap()
    wv = nc.dram_tensor("wv", [C, HPC * HD], F32R, kind="ExternalInput").ap()
    wo = nc.dram_tensor("wo", [HPC * HD, C], F32R, kind="ExternalInput").ap()
    mask = nc.dram_tensor("mask", [P, P], F32R, kind="ExternalInput").ap()
    ones = nc.dram_tensor("ones", [P, HD], F32R, kind="ExternalInput").ap()
    negshift = nc.dram_tensor("negshift", [P, 2 * P], F32R,
                              kind="ExternalInput").ap()
    y = nc.dram_tensor("y", [T, C], F32, kind="ExternalOutput").ap()

    with tile.TileContext(nc) as tc:
        with tc.tile_pool(name="sb", bufs=1) as sb, \
             tc.tile_pool(name="work", bufs=1) as work, \
             tc.tile_pool(name="dr", bufs=1, space="DRAM") as dr, \
             tc.tile_pool(name="ps", bufs=1, space="PSUM") as ps:

            # ---- static loads (wqk/xT first: they gate the first matmuls) ----
            wqk_sb = []
            wv_sb = []
            xT_sb = []
            for cb in range(CB):
                t_ = sb.tile([P, 2 * HPC * HD], F32R, tag=f"wqk{cb}", bufs=1,
                             name=f"wqk_sb{cb}")
                nc.sync.dma_start(t_, wqk[cb * P:(cb + 1) * P, :])
                wqk_sb.append(t_)
                t_ = sb.tile([P, T], F32R, tag=f"xT{cb}", bufs=1,
                             name=f"xT_sb{cb}")
                nc.scalar.dma_start(t_, xT[cb * P:(cb + 1) * P, :])
                xT_sb.append(t_)
            for cb in range(CB):
                t_ = sb.tile([P, HPC * HD], F32R, tag=f"wv{cb}", bufs=1,
                             name=f"wv_sb{cb}")
                nc.sync.dma_start(t_, wv[cb * P:(cb + 1) * P, :])
                wv_sb.append(t_)
            wo_sb = []
            for hp in range(2):
                t_ = sb.tile([P, C], F32R, tag=f"wo{hp}", bufs=1,
                             name=f"wo_sb{hp}")
                nc.sync.dma_start(t_, wo[hp * P:(hp + 1) * P, :])
                wo_sb.append(t_)
            ones_sb = sb.tile([P, HD], F32R, tag="ones", bufs=1)
            nc.sync.dma_start(ones_sb, ones)
            ones_row = ones_sb[0:1, :]
            mask_sb = sb.tile([P, P], F32R, tag="mask", bufs=1)
            nc.sync.dma_start(mask_sb, mask)
            negshift_sb = sb.tile([P, 2 * P], F32R, tag="negshift", bufs=1)
            nc.sync.dma_start(negshift_sb, negshift)
            # warm the exp table early (one tiny activation)
            exp_warm = sb.tile([1, HD], F32, tag="expwarm", bufs=1)
            nc.scalar.activation(exp_warm, ones_sb[0:1, :], EXP)

            # ---- qkv projection ----
            # qk transposed: qk_sb[jb] (128, T); jb 0,1 = q head pairs, 2,3 = k
            qk_sb = []
            for jb in range(4):
                t_ = sb.tile([P, T], F32R, tag=f"qk{jb}", bufs=1,
                             name=f"qk_sb{jb}")
                qk_sb.append(t_)
            # v natural per t-block, 4 heads x (64 v cols + ones col)
            v_sb = []
            for tb in range(NKB):
                t_ = sb.tile([P, HPC * (HD + 1)], F32R, tag=f"v{tb}", bufs=1,
                             name=f"v_sb{tb}")
                v_sb.append(t_)

            def qkv_units(tcg):
                """Yield PE-filler closures: one per qk psum group or v group."""
                tsl = slice(tcg * QCW, (tcg + 1) * QCW)

                def qk_unit(jb):
                    def emit():
                        ps_qk = ps.tile([P, QCW], F32, tag="ps", bufs=4,
                                        name="ps_qk")
                        for cb in range(CB):
                            nc.tensor.matmul(
                                ps_qk,
                                wqk_sb[cb][:, jb * P:(jb + 1) * P],
                                xT_sb[cb][:, tsl],
                                start=(cb == 0), stop=(cb == CB - 1))
                        nc.vector.tensor_copy(qk_sb[jb][:, tsl], ps_qk)
                    return emit

                def v_unit(tbl):
                    def emit():
                        tb = tcg * 4 + tbl
                        ps_v = ps.tile([P, HPC * HD], F32, tag="ps", bufs=4,
                                       name="ps_v")
                        for cb in range(CB):
                            nc.tensor.matmul(
                                ps_v,
                                xT_sb[cb][:, tb * P:(tb + 1) * P],
                                wv_sb[cb],
                                start=(cb == 0), stop=(cb == CB - 1))
                        vg = v_sb[tb].rearrange("p (h e) -> p h e", e=HD + 1)
                        nc.vector.tensor_copy(
                            vg[:, :, 0:HD],
                            ps_v.rearrange("p (h e) -> p h e", e=HD))
                        nc.vector.tensor_copy(
                            vg[:, :, HD:HD + 1],
                            ones_sb[:, 0:HPC].rearrange("p (h o) -> p h o", o=1))
                    return emit

                return [qk_unit(jb) for jb in range(4)] + \
                       [v_unit(tbl) for tbl in range(4)]

            def outproj_units(qc, attn):
                def op_unit(tbl):
                    def emit():
                        tb = qc * 4 + tbl
                        for cob in range(2):
                            out_sb = work.tile([P, QCW], F32, tag="outsb",
                                               bufs=4, name="out_sb")
                            ps_o = ps.tile([P, QCW], F32, tag="ps", bufs=4,
                                           name="ps_o")
                            for hp in range(2):
                                nc.tensor.matmul(
                                    ps_o,
                                    attn[hp][:, tbl * P:(tbl + 1) * P],
                                    wo_sb[hp][:, cob * QCW:(cob + 1) * QCW],
                                    start=(hp == 0), stop=(hp == 1))
                            if (tbl + cob) % 2 == 0:
                                nc.scalar.copy(out_sb, ps_o)
                            else:
                                nc.vector.tensor_copy(out_sb, ps_o)
                            nc.sync.dma_start(
                                y[tb * P:(tb + 1) * P,
                                  cob * QCW:(cob + 1) * QCW], out_sb)
                    return emit
                return [op_unit(tbl) for tbl in range(4)]

            def emit_attention(qc, filler):
                """scores -> fused exp -> AV, with PE filler interleaved to
                keep TensorE dense (HAM warm) while ACT chews the exps."""
                attn = {}
                avs = {}
                nkb = (qc + 1) * 4
                n_units = 2 * nkb
                fq = list(filler)
                credit = 0.0
                rate = len(fq) / n_units if n_units else 0.0
                ui = 0
                for hp in range(2):
                    at = work.tile([P, QCW], F32R, tag="attn", bufs=8,
                                   name=f"attn_hp{hp}")
                    attn[hp] = at
                    ps_av = {}
                    for par in range(2):
                        ps_av[par] = ps.tile([P, QCW], F32, tag="ps", bufs=4,
                                             name="ps_av")
                        avs[(hp, par)] = ps_av[par]
                    pend = []            # (kb, wexp2, jofs, w) awaiting AV
                    for kb in range(nkb):
                        kofs = kb - qc * 4
                        jofs = max(kofs, 0) * P
                        w = QCW - jofs
                        ps_s2 = ps.tile([P, 2 * QCW], F32, tag="ps2", bufs=2,
                                        name="ps_s2")
                        wexp2 = work.tile([P, 2 * QCW], F32R, tag="wexp2",
                                          bufs=4, name="wexp2")
                        diag = kofs >= 0
                        for par in range(2):
                            po = par * HD
                            nc.tensor.matmul(
                                ps_s2[:, par * QCW:par * QCW + w],
                                qk_sb[2 + hp][po:po + HD, kb * P:(kb + 1) * P],
                                qk_sb[hp][po:po + HD,
                                          qc * QCW + jofs:(qc + 1) * QCW],
                                start=True, stop=not diag)
                            if diag:
                                # accumulate -BIG onto the masked triangle:
                                # += mask.T @ negshift = -BIG * [i > j]
                                mw = min(2 * P, w)
                                nc.tensor.matmul(
                                    ps_s2[:, par * QCW:par * QCW + mw],
                                    mask_sb,
                                    negshift_sb[:, 0:mw],
                                    start=False, stop=True)
                        sview = ps_s2.rearrange("p (g q) -> p g q", g=2)
                        wview = wexp2.rearrange("p (g q) -> p g q", g=2)
                        nc.scalar.activation(wview[:, :, 0:w],
                                             sview[:, :, 0:w], EXP, scale=SCALE)
                        pend.append((kb, wexp2, jofs, w))
                        if len(pend) > 2:   # AV lags scores by 2 kb
                            _emit_av(hp, ps_av, pend.pop(0), nkb)
                        # drain PE filler to keep TensorE busy during exp
                        credit += rate
                        while credit >= 1.0 and fq:
                            fq.pop(0)()
                            credit -= 1.0
                        ui += 1
                    while pend:
                        _emit_av(hp, ps_av, pend.pop(0), nkb)
                        if fq:
                            fq.pop(0)()
                    # stage AV out of PSUM promptly so the banks recycle;
                    # heads land at their attn-aligned partition offsets
                    av_st = work.tile([P, QCW], F32, tag="avst",
                                      bufs=4, name="av_st")
                    sg = work.tile([2, QCW], F32, tag="sumg", bufs=2,
                                   name="sums_g")
                    s1 = work.tile([1, QCW], F32, tag="sum1", bufs=2,
                                   name="sums_1")
                    nc.vector.tensor_copy(av_st[0:HD, :], ps_av[0][0:HD, :])
                    nc.vector.tensor_copy(av_st[HD:P, :], ps_av[1][0:HD, :])
                    nc.vector.tensor_copy(sg[0:1, :], ps_av[0][HD:HD + 1, :])
                    nc.vector.tensor_copy(s1, ps_av[1][HD:HD + 1, :])
                    nc.sync.dma_start(sg[1:2, :], s1)   # gather row 1 via DMA
                    # chunked reciprocal+normalize: 128-col chunks so the
                    # out-projection per t-block unblocks as soon as its
                    # chunk is normalized (short tail)
                    rg = work.tile([2, QCW], F32R, tag="recg", bufs=2,
                                   name="rec_g")
                    for sc in range(4):
                        csl = slice(sc * P, (sc + 1) * P)
                        with nc.allow_low_precision(reason="f32r is fp32"):
                            nc.vector.reciprocal(rg[:, csl], sg[:, csl])
                        for par in range(2):
                            asl = attn[hp][par * HD:(par + 1) * HD, csl]
                            bounce = dr.tile([1, P], F32R, tag="bounce",
                                             bufs=8, name="bounce")
                            nc.sync.dma_start(bounce, rg[par:par + 1, csl])
                            nc.sync.dma_start(asl,
                                              bounce.to_broadcast([HD, P]))
                            nc.vector.tensor_mul(
                                asl, av_st[par * HD:(par + 1) * HD, csl],
                                asl)
                while fq:
                    fq.pop(0)()
                return attn, avs

            def _emit_av(hp, ps_av, pend, nkb):
                kb, wexp2, jofs, w = pend
                for par in range(2):
                    h = 2 * hp + par
                    nc.tensor.matmul(
                        ps_av[par][0:HD + 1, jofs:QCW],
                        v_sb[kb][:, h * (HD + 1):(h + 1) * (HD + 1)],
                        wexp2[:, par * QCW:par * QCW + w],
                        start=(kb == 0), stop=(kb == nkb - 1))

            # emission schedule: qkv(0) plain; attention(t) with qkv(t+1)
            # and outproj(t-1) interleaved as PE filler.
            for u in qkv_units(0):
                u()
            attns = []
            for tcg in range(NQC):
                if tcg < NQC - 1:
                    filler = qkv_units(tcg + 1)
                    if tcg == 1:
                        filler = filler + outproj_units(0, attns[0])
                else:
                    filler = (outproj_units(1, attns[1])
                              + outproj_units(2, attns[2]))
                attn, avs = emit_attention(tcg, filler)
                attns.append(attn)
            for u in outproj_units(NQC - 1, attns[3]):
                u()

    nc.compile()
    return nc


_PROGRAM = None


def _get_program():
    global _PROGRAM
    if _PROGRAM is None:
        _PROGRAM = build_program()
    return _PROGRAM


def make_in_maps(x, w_qkv, w_out):
    mask = np.triu(np.ones((P, P), dtype=np.float32))  # keep k<=q: i<=j
    # negshift[d, j] = -BIG iff d == j+1; mask.T @ negshift = -BIG*[i>j]
    negshift = np.zeros((P, 2 * P), dtype=np.float32)
    negshift[np.arange(1, P), np.arange(0, P - 1)] = -1e30
    in_maps = []
    for core in range(NCORES):
        b, p = core // HPC, core % HPC
        h0 = p * HPC * HD                       # first head col offset (256*p)
        in_maps.append({
            "xT": np.ascontiguousarray(x[b].T).astype(np.float32),
            "wqk": np.ascontiguousarray(np.concatenate(
                [w_qkv[:, h0:h0 + HPC * HD],
                 w_qkv[:, C + h0:C + h0 + HPC * HD]], axis=1)),
            "wv": np.ascontiguousarray(w_qkv[:, 2 * C + h0:2 * C + h0 + HPC * HD]),
            "wo": np.ascontiguousarray(w_out[h0:h0 + HPC * HD, :]),
            "mask": mask,
            "ones": np.ones((P, HD), dtype=np.float32),
            "negshift": negshift,
        })
    return in_maps


def kernel(x, w_qkv, w_out):
    x = np.asarray(x, dtype=np.float32)
    w_qkv = np.asarray(w_qkv, dtype=np.float32)
    w_out = np.asarray(w_out, dtype=np.float32)
    nc = _get_program()
    res = bass_utils.run_bass_kernel_spmd(nc, make_in_maps(x, w_qkv, w_out),
                                          core_ids=list(range(NCORES)))
    y = np.zeros((B, T, C), dtype=np.float32)
    for core in range(NCORES):
        y[core // HPC] += res.results[core]["y"]
    return y
